# revision 20
# baseline (speedup 1.0000x reference)
"""Trainium2 Bass kernel for nn_CrossModalAttentionBlock (GQA attention + top-2 MoE).

Strategy (8 NeuronCores, SPMD via run_bass_kernel_spmd):
  Phase 1 (token-parallel, fp32): each core takes one (batch, seq-half) shard of
    512 query tokens. It computes LN1, Q/K/V projections, GQA attention, output
    projection, residual, LN2 and the gate MLP entirely in the transposed
    [feature, token] domain (activations arrive pre-transposed from the host, so
    every matmul contracts over the partition dim with no on-device transposes).
    Outputs: x1^T (attention residual), h2^T (LN2 output), gate logits^T.
  Host: top-2 routing (softmax / top-k / renorm, mirroring the reference's
    semantics) + load-balanced packing of (expert, token-chunk) work units into
    2 FFN slots per core. Phase 1 runs in true fp32 because the gate top-2
    margins in this regime are tiny; fp32r-level error would flip routing
    decisions vs the reference.
  Phase 2 (slot-parallel, fp32r): each core runs 2 expert-FFN slots
    gelu(X@w1+b1)@w2+b2 scaled by the renormalized gate weight, on tokens/weights
    the host assigned to it. fp32r runs the PE at 4x the fp32 rate and its
    ~1e-4 rounding only perturbs the (additive) MoE branch, not routing.
  Host: scatter-add slot outputs + final residual.
"""

import numpy as np

import concourse.bass as bass
import concourse.mybir as mybir
import concourse.tile as tile
from concourse import bacc
from concourse.bass_utils import run_bass_kernel_spmd

AF = mybir.ActivationFunctionType
ALU = mybir.AluOpType
FP32 = mybir.dt.float32
FP32R = mybir.dt.float32r

B, S, D = 4, 1024, 1024
H, G = 16, 8
HD = D // H              # 64
E, TOPK, ED = 8, 2, 2 * D
GH = D // 2              # 512
EPS = 1e-5
P = 128
NCORES = 8
SQ = S // 2              # 512 query tokens per core
T = B * S
DC = D // P              # 8 feature chunks
SCALE = HD ** -0.5


# ---------------------------------------------------------------- phase 1 ----

def _stats_rows(nc, ps_stat, stats, ones_col, load_half, sq_pool, width, ones_r=None):
    """Column stats over the D (partition x 8-chunk) axis of a [D, width]
    transposed activation. load_half(kd, n, w) returns a [128, w] AP for chunk
    kd, column window n*512..n*512+w. Returns (a_row, c_row) [1, width] rows on
    partition 0 with a = rsqrt(var+eps), c = -mu*a."""
    nn = (width + 511) // 512
    sum_sb = stats.tile([1, width], FP32, tag="stat_rows")
    sq_sb = stats.tile([1, width], FP32, tag="stat_rows")
    for n in range(nn):
        w = min(512, width - n * 512)
        psx = ps_stat.tile([1, 512], FP32, tag="ps_stat")
        psq = ps_stat.tile([1, 512], FP32, tag="ps_stat")
        for kd in range(DC):
            t = load_half(kd, n, w)
            nc.tensor.matmul(psx[:, :w], ones_col[:], t,
                             start=(kd == 0), stop=(kd == DC - 1))
            sq = sq_pool.tile([128, 512], FP32R, tag="sq_tmp_r")
            nc.scalar.activation(sq[:, :w], t, AF.Square)
            nc.tensor.matmul(psq[:, :w], ones_r[:], sq[:, :w],
                             start=(kd == 0), stop=(kd == DC - 1))
        nc.scalar.activation(sum_sb[:, n * 512:n * 512 + w], psx[:, :w], AF.Copy, scale=1.0 / D)
        nc.scalar.activation(sq_sb[:, n * 512:n * 512 + w], psq[:, :w], AF.Copy, scale=1.0 / D)
    # var = E[x^2] - mu^2 ; a = 1/sqrt(var+eps) ; c = -mu*a
    a_row = stats.tile([1, width], FP32, tag="stat_rows")
    c_row = stats.tile([1, width], FP32, tag="stat_rows")
    nc.vector.tensor_tensor(c_row[:], sum_sb[:], sum_sb[:], ALU.mult)        # mu^2
    nc.vector.tensor_tensor(sq_sb[:], sq_sb[:], c_row[:], ALU.subtract)      # var
    nc.vector.tensor_scalar_add(sq_sb[:], sq_sb[:], EPS)
    nc.scalar.activation(c_row[:], sq_sb[:], AF.Sqrt)                        # sd
    nc.vector.reciprocal(a_row[:], c_row[:])
    nc.vector.tensor_tensor(c_row[:], sum_sb[:], a_row[:], ALU.mult)         # mu*a
    nc.scalar.activation(c_row[:], c_row[:], AF.Copy, scale=-1.0)            # -mu*a
    return a_row, c_row


def _ln_apply(nc, bcast_pool, dst_tiles, src_tiles, a_row, c_row, g_pc, b_pc, width):
    """dst[kd] = (src[kd]*a + c)*g[kd] + b[kd] with a,c rows broadcast across
    partitions (gpsimd) and g,b per-partition scalars."""
    a_b = bcast_pool.tile([128, width], FP32, tag="ln_bcast")
    c_b = bcast_pool.tile([128, width], FP32, tag="ln_bcast")
    nc.gpsimd.partition_broadcast(a_b[:], a_row[:])
    nc.gpsimd.partition_broadcast(c_b[:], c_row[:])
    for kd in range(DC):
        nc.vector.tensor_tensor(dst_tiles[kd][:], src_tiles[kd][:], a_b[:], ALU.mult)
        nc.vector.tensor_tensor(dst_tiles[kd][:], dst_tiles[kd][:], c_b[:], ALU.add)
        nc.scalar.activation(dst_tiles[kd][:], dst_tiles[kd][:], AF.Identity,
                             bias=b_pc[:, kd:kd + 1], scale=g_pc[:, kd:kd + 1])


def build_phase1():
    nc = bacc.Bacc("TRN2", target_bir_lowering=False, debug=False, num_devices=NCORES)

    xbT_d = nc.dram_tensor("xbT", [D, S], FP32R, kind="ExternalInput").ap()
    xqT_d = nc.dram_tensor("xqT", [D, SQ], FP32, kind="ExternalInput").ap()
    wq_d = nc.dram_tensor("wq", [D, G * HD], FP32R, kind="ExternalInput").ap()
    wk_d = nc.dram_tensor("wk", [D, D], FP32R, kind="ExternalInput").ap()
    wv_d = nc.dram_tensor("wv", [D, D], FP32R, kind="ExternalInput").ap()
    wo_d = nc.dram_tensor("wo", [D, D], FP32R, kind="ExternalInput").ap()
    bq_d = nc.dram_tensor("bq", [G * HD], FP32, kind="ExternalInput").ap()
    bk_d = nc.dram_tensor("bk", [D], FP32, kind="ExternalInput").ap()
    bv_d = nc.dram_tensor("bv", [D], FP32, kind="ExternalInput").ap()
    bo_d = nc.dram_tensor("bo", [D], FP32, kind="ExternalInput").ap()
    ln1g_d = nc.dram_tensor("ln1g", [D], FP32, kind="ExternalInput").ap()
    ln1b_d = nc.dram_tensor("ln1b", [D], FP32, kind="ExternalInput").ap()
    ln2g_d = nc.dram_tensor("ln2g", [D], FP32, kind="ExternalInput").ap()
    ln2b_d = nc.dram_tensor("ln2b", [D], FP32, kind="ExternalInput").ap()
    gw1_d = nc.dram_tensor("gw1", [D, GH], FP32R, kind="ExternalInput").ap()
    gb1_d = nc.dram_tensor("gb1", [GH], FP32, kind="ExternalInput").ap()
    gw2_d = nc.dram_tensor("gw2", [GH, E], FP32R, kind="ExternalInput").ap()
    gb2_d = nc.dram_tensor("gb2", [E], FP32, kind="ExternalInput").ap()

    x1T_d = nc.dram_tensor("x1T", [D, SQ], FP32, kind="ExternalOutput").ap()
    h2T_d = nc.dram_tensor("h2T", [D, SQ], FP32, kind="ExternalOutput").ap()
    glogT_d = nc.dram_tensor("glogT", [E, SQ], FP32, kind="ExternalOutput").ap()

    with tile.TileContext(nc) as tc:
        import contextlib
        ctx = contextlib.ExitStack()
        with ctx:
            const = ctx.enter_context(tc.tile_pool(name="const", bufs=1))
            stats = ctx.enter_context(tc.tile_pool(name="stats", bufs=4))
            bcast = ctx.enter_context(tc.tile_pool(name="bcast", bufs=2))
            sq_pool = ctx.enter_context(tc.tile_pool(name="sq", bufs=2))
            wslice = ctx.enter_context(tc.tile_pool(name="wslice", bufs=2))
            qt_pool = ctx.enter_context(tc.tile_pool(name="qt", bufs=4))
            xq_pool = ctx.enter_context(tc.tile_pool(name="xq", bufs=DC))
            ut_pool = ctx.enter_context(tc.tile_pool(name="ut", bufs=DC))
            ps_big = ctx.enter_context(tc.tile_pool(name="psb", bufs=3, space="PSUM"))
            ps_ao = ctx.enter_context(tc.tile_pool(name="psao", bufs=3, space="PSUM"))
            ps_stat = ctx.enter_context(tc.tile_pool(name="psst", bufs=2, space="PSUM"))

            ones_col = const.tile([128, 1], FP32)
            nc.vector.memset(ones_col[:], 1.0)
            ones_col_r = const.tile([128, 1], FP32R)
            nc.scalar.copy(ones_col_r[:], ones_col[:])

            def pc_load(nm, src, ncol):  # [ncol*128] dram -> [128, ncol] sbuf
                t = const.tile([128, ncol], FP32, tag=f"pc{nm}", name=f"pc{nm}")
                nc.sync.dma_start(t[:], src.rearrange("(c p) -> p c", p=128))
                return t

            g1_pc = pc_load("g1", ln1g_d, DC)
            b1_pc = pc_load("b1", ln1b_d, DC)
            g2_pc = pc_load("g2", ln2g_d, DC)
            b2_pc = pc_load("b2", ln2b_d, DC)
            bk_pc = pc_load("bk", bk_d, DC)
            bo_pc = pc_load("bo", bo_d, DC)
            bq_pc = pc_load("bq", bq_d, 4)
            gb1_pc = pc_load("gb1", gb1_d, 4)
            gb2_pc = const.tile([E, 1], FP32)
            nc.sync.dma_start(gb2_pc[:], gb2_d[:, None])
            bv_row = const.tile([1, D], FP32)
            nc.sync.dma_start(bv_row[:], bv_d[None, :])
            bv_b = const.tile([128, D], FP32)
            nc.gpsimd.partition_broadcast(bv_b[:], bv_row[:])
            ones16 = const.tile([128, 16], FP32)
            nc.vector.memset(ones16[:], 1.0)
            zro = const.tile([128, SQ], FP32)
            nc.vector.memset(zro[:], 0.0)

            # xqT stays resident until the attention residual.
            xqT = [xq_pool.tile([128, SQ], FP32, tag="xqT", name=f"xqT{i}") for i in range(DC)]
            for kd in range(DC):
                nc.sync.dma_start(xqT[kd][:], xqT_d[kd * 128:(kd + 1) * 128, :])

            # ---- hqT = LN1(xqT) -> QT ----------------------------------
            QT = [qt_pool.tile([128, SQ], FP32R, tag="QT", name=f"QT{i}") for i in range(4)]
            with tc.tile_pool(name="hq", bufs=DC) as hq_pool:
                aq, cq = _stats_rows(nc, ps_stat, stats, ones_col,
                                     lambda kd, n, w: xqT[kd][:, n * 512:n * 512 + w], sq_pool, SQ,
                                     ones_r=ones_col_r)
                hqT = [hq_pool.tile([128, SQ], FP32R, tag="hqT", name=f"hqT{i}") for i in range(DC)]
                _ln_apply(nc, bcast, hqT, xqT, aq, cq, g1_pc, b1_pc, SQ)
                wqr = wq_d.rearrange("(c p) m -> p c m", p=128)
                for m in range(4):  # QT chunk m: groups 2m (rows 0:64), 2m+1 (64:128)
                    wqb = wslice.tile([128, DC, 128], FP32R, tag="wsl")
                    nc.sync.dma_start(wqb[:], wqr[:, :, m * 128:(m + 1) * 128])
                    psq = ps_big.tile([128, 512], FP32, tag="ps_big")
                    for kd in range(DC):
                        nc.tensor.matmul(psq[:], wqb[:, kd, :], hqT[kd][:], start=(kd == 0), stop=(kd == DC - 1))
                    nc.scalar.activation(QT[m][:], psq[:], AF.Identity, bias=bq_pc[:, m:m + 1])

            # ---- hT = LN1(xbT) full sequence, then K^T and V ------------
            kt_pool = ctx.enter_context(tc.tile_pool(name="kt", bufs=DC))
            va_pool = ctx.enter_context(tc.tile_pool(name="va", bufs=DC))
            KT = [kt_pool.tile([128, S], FP32R, tag="KT", name=f"KT{i}") for i in range(DC)]
            V_aug = [va_pool.tile([128, 16, 65], FP32R, tag="V_aug", name=f"V_aug{i}") for i in range(DC)]
            with tc.tile_pool(name="h", bufs=DC) as h_pool:
                hT = [h_pool.tile([128, S], FP32R, tag="hT", name=f"hT{i}") for i in range(DC)]
                with (
                    tc.tile_pool(name="xbh", bufs=3) as xbh_pool,
                    tc.tile_pool(name="xbf", bufs=2) as xb_pool,
                ):
                    def load_xb_half(kd, n, w):
                        t = xbh_pool.tile([128, 512], FP32R, tag="xb_half")
                        nc.sync.dma_start(t[:, :w], xbT_d[kd * 128:(kd + 1) * 128,
                                                          n * 512:n * 512 + w])
                        return t[:, :w]
                    ab, cb = _stats_rows(nc, ps_stat, stats, ones_col_r, load_xb_half, sq_pool, S,
                                         ones_r=ones_col_r)
                    a_b = bcast.tile([128, S], FP32, tag="ln_bcast")
                    c_b = bcast.tile([128, S], FP32, tag="ln_bcast")
                    nc.gpsimd.partition_broadcast(a_b[:], ab[:])
                    nc.gpsimd.partition_broadcast(c_b[:], cb[:])
                    for kd in range(DC):
                        xt = xb_pool.tile([128, S], FP32R, tag="xb_full")
                        nc.sync.dma_start(xt[:], xbT_d[kd * 128:(kd + 1) * 128, :])
                        nc.vector.tensor_tensor(hT[kd][:], xt[:], a_b[:], ALU.mult)
                        nc.vector.tensor_tensor(hT[kd][:], hT[kd][:], c_b[:], ALU.add)
                        nc.scalar.activation(hT[kd][:], hT[kd][:], AF.Identity,
                                             bias=b1_pc[:, kd:kd + 1], scale=g1_pc[:, kd:kd + 1])

                wkr = wk_d.rearrange("(c p) m -> p c m", p=128)
                for m in range(DC):
                    wkb = wslice.tile([128, DC, 128], FP32R, tag="wsl")
                    nc.sync.dma_start(wkb[:], wkr[:, :, m * 128:(m + 1) * 128])
                    for n in range(2):
                        psk = ps_big.tile([128, 512], FP32, tag="ps_big")
                        for kd in range(DC):
                            nc.tensor.matmul(psk[:], wkb[:, kd, :], hT[kd][:, n * 512:(n + 1) * 512],
                                             start=(kd == 0), stop=(kd == DC - 1))
                        nc.scalar.activation(KT[m][:, n * 512:(n + 1) * 512], psk[:],
                                             AF.Identity, bias=bk_pc[:, m:m + 1])

                with tc.tile_pool(name="wvp", bufs=DC) as wv_pool:
                    for sc in range(DC):
                        nc.scalar.copy(V_aug[sc][:, :, 64:65].rearrange("p h one -> p (h one)"), ones16[:])
                    for n in range(2):
                        wv_sb = [wv_pool.tile([128, 512], FP32R, tag="wv_sb", name=f"wv_sb{n}_{i}")
                                 for i in range(DC)]
                        for kd in range(DC):
                            nc.sync.dma_start(wv_sb[kd][:], wv_d[kd * 128:(kd + 1) * 128,
                                                                 n * 512:(n + 1) * 512])
                        for sc in range(DC):
                            psv = ps_big.tile([128, 512], FP32, tag="ps_big")
                            for kd in range(DC):
                                nc.tensor.matmul(psv[:], hT[kd][:, sc * 128:(sc + 1) * 128],
                                                 wv_sb[kd][:],
                                                 start=(kd == 0), stop=(kd == DC - 1))
                            nc.vector.tensor_tensor(
                                V_aug[sc][:, n * 8:(n + 1) * 8, 0:64],
                                psv.rearrange("p (h d) -> p h d", d=64),
                                bv_b[:, n * 512:(n + 1) * 512].rearrange("p (h d) -> p h d", d=64),
                                ALU.add)

            # ---- attention per head -------------------------------------
            UT = [ut_pool.tile([128, SQ], FP32R, tag="UT", name=f"UT{i}") for i in range(DC)]
            with (
                tc.tile_pool(name="rq", bufs=3) as rq_pool,
                tc.tile_pool(name="ex", bufs=DC + 1) as ex_pool,
                tc.tile_pool(name="den", bufs=2) as den_pool,
            ):
                for h in range(H):
                    pair = h // 2
                    m, src_off, dst_off = pair // 2, (pair % 2) * 64, (h % 2) * 64
                    rq = rq_pool.tile([128, SQ], FP32R, tag="rhsQ")
                    zoff = 64 - dst_off
                    nc.scalar.copy(rq[zoff:zoff + 64, :], zro[zoff:zoff + 64, :])
                    if src_off == dst_off:
                        nc.vector.tensor_copy(rq[dst_off:dst_off + 64, :],
                                              QT[m][src_off:src_off + 64, :])
                    else:
                        nc.sync.dma_start(rq[dst_off:dst_off + 64, :],
                                          QT[m][src_off:src_off + 64, :])
                    expS = [ex_pool.tile([128, SQ], FP32R, tag="expS", name=f"expS{h}_{i}")
                            for i in range(DC)]
                    for kc in range(DC):
                        pss = ps_big.tile([128, 512], FP32, tag="ps_big")
                        nc.tensor.matmul(pss[:], KT[pair][:, kc * 128:(kc + 1) * 128], rq[:],
                                         start=True, stop=True)
                        nc.scalar.activation(expS[kc][:], pss[:], AF.Exp, scale=SCALE)
                    psa = ps_ao.tile([65, 512], FP32, tag="ps_ao")
                    for kc in range(DC):
                        nc.tensor.matmul(psa[:], V_aug[kc][:, h, :], expS[kc][:],
                                         start=(kc == 0), stop=(kc == DC - 1))
                    aosb = den_pool.tile([65, 512], FP32, tag="aosb")
                    nc.scalar.copy(aosb[:], psa[:])
                    den0 = den_pool.tile([1, 512], FP32, tag="den0")
                    nc.sync.dma_start(den0[:], aosb[64:65, :])
                    rec0 = den_pool.tile([1, 512], FP32, tag="rec0")
                    nc.vector.reciprocal(rec0[:], den0[:])
                    recb = den_pool.tile([64, 512], FP32, tag="recb")
                    nc.gpsimd.partition_broadcast(recb[:], rec0[:])
                    if h % 2 == 0:
                        nc.vector.tensor_tensor(UT[pair][0:64, :], aosb[0:64, :], recb[:], ALU.mult)
                    else:
                        aostage = den_pool.tile([64, 512], FP32R, tag="aostage")
                        nc.vector.tensor_tensor(aostage[:], aosb[0:64, :], recb[:], ALU.mult)
                        nc.sync.dma_start(UT[pair][64:128, :], aostage[:])

            # ---- output projection + residual + LN2 + gate --------------
            with (
                tc.tile_pool(name="x1", bufs=DC) as x1_pool,
                tc.tile_pool(name="h2", bufs=DC) as h2_pool,
                tc.tile_pool(name="gh", bufs=4) as gh_pool,
            ):
                x1T = [x1_pool.tile([128, SQ], FP32, tag="x1T", name=f"x1T{i}") for i in range(DC)]
                wor = wo_d.rearrange("(c p) m -> p c m", p=128)
                for m in range(DC):
                    wob = wslice.tile([128, DC, 128], FP32R, tag="wsl")
                    nc.sync.dma_start(wob[:], wor[:, :, m * 128:(m + 1) * 128])
                    pso = ps_big.tile([128, 512], FP32, tag="ps_big")
                    for pr in range(DC):
                        nc.tensor.matmul(pso[:], wob[:, pr, :], UT[pr][:], start=(pr == 0), stop=(pr == DC - 1))
                    nc.scalar.activation(x1T[m][:], pso[:], AF.Identity, bias=bo_pc[:, m:m + 1])
                    nc.vector.tensor_tensor(x1T[m][:], x1T[m][:], xqT[m][:], ALU.add)
                    nc.sync.dma_start(x1T_d[m * 128:(m + 1) * 128, :], x1T[m][:])

                a2, c2 = _stats_rows(nc, ps_stat, stats, ones_col,
                                     lambda kd, n, w: x1T[kd][:, n * 512:n * 512 + w], sq_pool, SQ,
                                     ones_r=ones_col_r)
                h2T = [h2_pool.tile([128, SQ], FP32R, tag="h2T", name=f"h2T{i}") for i in range(DC)]
                _ln_apply(nc, bcast, h2T, x1T, a2, c2, g2_pc, b2_pc, SQ)
                for m in range(DC):
                    nc.sync.dma_start(h2T_d[m * 128:(m + 1) * 128, :], h2T[m].bitcast(FP32)[:])

                GhT = [gh_pool.tile([128, SQ], FP32R, tag="GhT", name=f"GhT{i}") for i in range(4)]
                gw1r = gw1_d.rearrange("(c p) m -> p c m", p=128)
                for m in range(4):
                    gwb = wslice.tile([128, DC, 128], FP32R, tag="wsl")
                    nc.sync.dma_start(gwb[:], gw1r[:, :, m * 128:(m + 1) * 128])
                    psg = ps_big.tile([128, 512], FP32, tag="ps_big")
                    for kd in range(DC):
                        nc.tensor.matmul(psg[:], gwb[:, kd, :], h2T[kd][:], start=(kd == 0), stop=(kd == DC - 1))
                    nc.scalar.activation(GhT[m][:], psg[:], AF.Relu, bias=gb1_pc[:, m:m + 1])
                psl = ps_ao.tile([E, 512], FP32, tag="ps_ao")
                for gt in range(4):
                    wsl = wslice.tile([128, E], FP32R, tag="wsl_g2")
                    nc.sync.dma_start(wsl[:], gw2_d[gt * 128:(gt + 1) * 128, :])
                    nc.tensor.matmul(psl[:], wsl[:], GhT[gt][:], start=(gt == 0), stop=(gt == 3))
                glog_sb = stats.tile([E, SQ], FP32, tag="glog")
                nc.scalar.activation(glog_sb[:], psl[:], AF.Identity, bias=gb2_pc[:])
                nc.sync.dma_start(glogT_d[:], glog_sb[:])

    nc.compile()
    return nc


# ---------------------------------------------------------------- phase 2 ----

def build_phase2(caps):
    """One FFN slot per capacity in `caps` (same shapes on all cores)."""
    nc = bacc.Bacc("TRN2", target_bir_lowering=False, debug=False, num_devices=NCORES)
    ins = []
    outs = []
    for si, C in enumerate(caps):
        ins.append(dict(
            xgT=nc.dram_tensor(f"xg{si}T", [D, C], FP32R, kind="ExternalInput").ap(),
            w1=nc.dram_tensor(f"w1_{si}", [D, ED], FP32R, kind="ExternalInput").ap(),
            w2=nc.dram_tensor(f"w2_{si}", [ED, D], FP32R, kind="ExternalInput").ap(),
            b1=nc.dram_tensor(f"b1_{si}", [ED], FP32, kind="ExternalInput").ap(),
            b2=nc.dram_tensor(f"b2_{si}", [D], FP32, kind="ExternalInput").ap(),
            wt=nc.dram_tensor(f"wt{si}", [1, C], FP32, kind="ExternalInput").ap(),
        ))
        outs.append(nc.dram_tensor(f"y{si}T", [D, C], FP32, kind="ExternalOutput").ap())

    EC = ED // P  # 16 hidden chunks

    with tile.TileContext(nc) as tc:
        import contextlib
        ctx = contextlib.ExitStack()
        with ctx:
            const = ctx.enter_context(tc.tile_pool(name="const", bufs=1))
            xg_pool = ctx.enter_context(tc.tile_pool(name="xg", bufs=DC + 1))
            hid_pool = ctx.enter_context(tc.tile_pool(name="hid", bufs=EC + 1))
            wsl_pool = ctx.enter_context(tc.tile_pool(name="wsl", bufs=3))
            out_pool = ctx.enter_context(tc.tile_pool(name="out", bufs=3))
            misc = ctx.enter_context(tc.tile_pool(name="misc", bufs=4))
            ps = ctx.enter_context(tc.tile_pool(name="ps", bufs=4, space="PSUM"))

            for si, C in enumerate(caps):
                io = ins[si]
                NN = [min(512, C - n * 512) for n in range((C + 511) // 512)]
                b1_pc = misc.tile([128, EC], FP32, tag="b1pc")
                nc.sync.dma_start(b1_pc[:], io["b1"].rearrange("(c p) -> p c", p=128))
                b2_pc = misc.tile([128, DC], FP32, tag="b2pc")
                nc.sync.dma_start(b2_pc[:], io["b2"].rearrange("(c p) -> p c", p=128))
                wt_row = misc.tile([1, C], FP32, tag="wtrow")
                nc.sync.dma_start(wt_row[:], io["wt"][:])
                wt_b = misc.tile([128, C], FP32, tag="wtb")
                nc.gpsimd.partition_broadcast(wt_b[:], wt_row[:])

                xgT = [xg_pool.tile([128, C], FP32R, tag="xgT", name=f"xgT{si}_{i}") for i in range(DC)]
                for kd in range(DC):
                    nc.sync.dma_start(xgT[kd][:], io["xgT"][kd * 128:(kd + 1) * 128, :])

                hidT = [hid_pool.tile([128, C], FP32R, tag="hidT", name=f"hidT{si}_{i}") for i in range(EC)]
                w1r = io["w1"].rearrange("(c p) m -> p c m", p=128)
                for ec in range(EC):
                    w1b = wsl_pool.tile([128, DC, 128], FP32R, tag="w1sl")
                    nc.sync.dma_start(w1b[:], w1r[:, :, ec * 128:(ec + 1) * 128])
                    wsls = [w1b[:, kd, :] for kd in range(DC)]
                    for n, nn_w in enumerate(NN):
                        psh = ps.tile([128, 512], FP32, tag="psh")
                        for kd in range(DC):
                            nc.tensor.matmul(psh[:, :nn_w], wsls[kd],
                                             xgT[kd][:, n * 512:n * 512 + nn_w],
                                             start=(kd == 0), stop=(kd == DC - 1))
                        nc.scalar.activation(hidT[ec][:, n * 512:n * 512 + nn_w], psh[:, :nn_w],
                                             AF.Gelu, bias=b1_pc[:, ec:ec + 1])

                w2r = io["w2"].rearrange("(c p) m -> p c m", p=128)
                for m in range(DC):
                    w2b = wsl_pool.tile([128, EC, 128], FP32R, tag="w2sl")
                    nc.sync.dma_start(w2b[:], w2r[:, :, m * 128:(m + 1) * 128])
                    wsls = [w2b[:, et, :] for et in range(EC)]
                    ostage = out_pool.tile([128, C], FP32, tag="ostage")
                    for n, nn_w in enumerate(NN):
                        psy = ps.tile([128, 512], FP32, tag="psy")
                        for et in range(EC):
                            nc.tensor.matmul(psy[:, :nn_w], wsls[et],
                                             hidT[et][:, n * 512:n * 512 + nn_w],
                                             start=(et == 0), stop=(et == EC - 1))
                        nc.scalar.activation(ostage[:, n * 512:n * 512 + nn_w], psy[:, :nn_w],
                                             AF.Identity, bias=b2_pc[:, m:m + 1])
                    nc.vector.tensor_tensor(ostage[:], ostage[:], wt_b[:], ALU.mult)
                    nc.sync.dma_start(outs[si][m * 128:(m + 1) * 128, :], ostage[:])

    nc.compile()
    return nc


# ------------------------------------------------------------------- host ----

_CACHE = {}


def _softmax_np(x, axis=-1):
    m = x.max(axis=axis, keepdims=True)
    e = np.exp(x - m)
    return e / e.sum(axis=axis, keepdims=True)


def _pack_slots(tok_lists, wt_lists):
    """Pack per-expert token lists into 16 slots (2 per core) balancing load.
    Returns (caps, assignment) where assignment[core] = [(expert, toks, wts), ...]."""
    menu = [(512, 512), (768, 512), (1024, 512), (1024, 768), (1280, 1024),
            (1536, 1024), (1536, 1536)]
    for c1, c2 in menu:
        bigs, smalls = [], []
        ok = True
        for e in range(E):
            t, w = tok_lists[e], wt_lists[e]
            while len(t) > c2:
                take = min(c1, len(t))
                bigs.append((e, t[:take], w[:take]))
                t, w = t[take:], w[take:]
            if len(t):
                smalls.append((e, t, w))
        if len(bigs) <= NCORES and len(smalls) <= NCORES:
            caps = (c1, c2)
            # sort desc and pair big-desc with small-asc for balance
            bigs.sort(key=lambda u: -len(u[1]))
            smalls.sort(key=lambda u: len(u[1]))
            while len(bigs) < NCORES:
                bigs.append((0, np.zeros(0, np.int64), np.zeros(0, np.float32)))
            while len(smalls) < NCORES:
                smalls.append((0, np.zeros(0, np.int64), np.zeros(0, np.float32)))
            assignment = [[bigs[c], smalls[c]] for c in range(NCORES)]
            return caps, assignment
    raise RuntimeError("no slot config fits")



def _exact_gate_rows(x, wq, bq, wk, bk, wv, bv, wo, bo, ln1g, ln1b, ln2g, ln2b,
                     gw1, gb1, gw2, gb2, toks):
    """Recompute gate logits for the given flat token ids in float64 (K/V gemms
    in float32), mirroring the reference pipeline. Used only to pin down top-2
    routing for tokens whose gate margin is within the device error."""
    f8 = np.float64
    out = np.zeros((len(toks), E), f8)
    byb = {}
    for i, t in enumerate(toks):
        byb.setdefault(int(t) // S, []).append((i, int(t) % S))
    for b, items in byb.items():
        xb = x[b].astype(f8)
        mu = xb.mean(1, keepdims=True)
        va = xb.var(1, keepdims=True)
        h = (xb - mu) / np.sqrt(va + EPS) * ln1g + ln1b
        h32 = h.astype(np.float32)
        K = h32 @ wk + bk
        V = h32 @ wv + bv
        for i, s in items:
            q = h[s] @ wq.astype(f8) + bq
            ao = np.empty(D, f8)
            for hh in range(H):
                g = hh // 2
                sc = (K[:, hh * HD:(hh + 1) * HD].astype(f8) @ q[g * HD:(g + 1) * HD]) * SCALE
                p = np.exp(sc - sc.max())
                p /= p.sum()
                ao[hh * HD:(hh + 1) * HD] = p @ V[:, hh * HD:(hh + 1) * HD].astype(f8)
            x1 = x[b, s].astype(f8) + ao @ wo.astype(f8) + bo
            h2 = (x1 - x1.mean()) / np.sqrt(x1.var() + EPS) * ln2g + ln2b
            gl = np.maximum(h2 @ gw1.astype(f8) + gb1, 0.0) @ gw2.astype(f8) + gb2
            out[i] = gl
    return out


def kernel(**inputs):
    x = np.asarray(inputs["x"], np.float32)
    get = lambda k: np.ascontiguousarray(np.asarray(inputs[k], np.float32))
    wq, wk, wv, wo = get("wq"), get("wk"), get("wv"), get("wo")
    bq, bk, bv, bo = get("bq"), get("bk"), get("bv"), get("bo")
    ln1g, ln1b, ln2g, ln2b = get("ln1_g"), get("ln1_b"), get("ln2_g"), get("ln2_b")
    gw1, gb1, gw2, gb2 = get("gw1"), get("gb1"), get("gw2"), get("gb2")
    ew1, eb1, ew2, eb2 = get("ew1"), get("eb1"), get("ew2"), get("eb2")

    if "p1" not in _CACHE:
        _CACHE["p1"] = build_phase1()
    nc1 = _CACHE["p1"]

    shared = dict(wq=wq, wk=wk, wv=wv, wo=wo, bq=bq, bk=bk, bv=bv, bo=bo,
                  ln1g=ln1g, ln1b=ln1b, ln2g=ln2g, ln2b=ln2b,
                  gw1=gw1, gb1=gb1, gw2=gw2, gb2=gb2)
    in_maps = []
    for c in range(NCORES):
        b, half = c // 2, c % 2
        xbT = np.ascontiguousarray(x[b].T)
        xqT = np.ascontiguousarray(xbT[:, half * SQ:(half + 1) * SQ])
        in_maps.append(dict(shared, xbT=xbT, xqT=xqT))
    r1 = run_bass_kernel_spmd(nc1, in_maps, core_ids=list(range(NCORES)))

    x1 = np.empty((T, D), np.float32)
    h2 = np.empty((T, D), np.float32)
    glog = np.empty((T, E), np.float32)
    for c in range(NCORES):
        b, half = c // 2, c % 2
        sl = slice(b * S + half * SQ, b * S + (half + 1) * SQ)
        x1[sl] = r1.results[c]["x1T"].T
        h2[sl] = r1.results[c]["h2T"].T
        glog[sl] = r1.results[c]["glogT"].T

    # top-2 routing, mirroring the reference: softmax -> top_k -> softmax renorm.
    # Tokens whose 2nd/3rd gate margin is inside the device error envelope get
    # their logits recomputed exactly on the host so selection matches the
    # reference bit-for-bit.
    gate_w = _softmax_np(glog)
    srt = np.sort(gate_w, axis=1)
    sus = np.where(srt[:, -2] - srt[:, -3] < 5e-4)[0]
    if len(sus):
        glog[sus] = _exact_gate_rows(
            x, wq, bq, wk, bk, wv, bv, wo, bo, ln1g, ln1b, ln2g, ln2b,
            gw1, gb1, gw2, gb2, sus).astype(np.float32)
        gate_w[sus] = _softmax_np(glog[sus])
    idx = np.argsort(-gate_w, axis=1, kind="stable")[:, :TOPK]
    top_w = np.take_along_axis(gate_w, idx, axis=1)
    ren = _softmax_np(top_w)

    tok_lists, wt_lists = [], []
    for e in range(E):
        sel0 = np.where(idx[:, 0] == e)[0]
        sel1 = np.where(idx[:, 1] == e)[0]
        tok_lists.append(np.concatenate([sel0, sel1]))
        wt_lists.append(np.concatenate([ren[sel0, 0], ren[sel1, 1]]).astype(np.float32))

    caps, assignment = _pack_slots(tok_lists, wt_lists)
    if ("p2", caps) not in _CACHE:
        _CACHE[("p2", caps)] = build_phase2(caps)
    nc2 = _CACHE[("p2", caps)]

    in_maps2 = []
    for c in range(NCORES):
        m = {}
        for si, (e, toks, wts) in enumerate(assignment[c]):
            C = caps[si]
            xgT = np.zeros((D, C), np.float32)
            if len(toks):
                xgT[:, :len(toks)] = h2[toks].T
            wt = np.zeros((1, C), np.float32)
            wt[0, :len(toks)] = wts
            m[f"xg{si}T"] = xgT
            m[f"w1_{si}"] = ew1[e]
            m[f"w2_{si}"] = ew2[e]
            m[f"b1_{si}"] = eb1[e]
            m[f"b2_{si}"] = eb2[e]
            m[f"wt{si}"] = wt
        in_maps2.append(m)
    r2 = run_bass_kernel_spmd(nc2, in_maps2, core_ids=list(range(NCORES)))

    moe = np.zeros((T, D), np.float32)
    for c in range(NCORES):
        for si, (e, toks, wts) in enumerate(assignment[c]):
            if len(toks):
                # token indices are unique within a slot, so fancy += is safe
                moe[toks] += r2.results[c][f"y{si}T"][:, :len(toks)].T

    return (x1 + moe).reshape(B, S, D).astype(np.float32)
